# revision 1
# baseline (speedup 1.0000x reference)
"""3-layer GAT on 8 Trainium2 NeuronCores (Bass/Tile SPMD).

Sharding: nodes partitioned into 8 contiguous blocks of 6250 (padded to 6272);
edges assigned to the core owning their dst node, so per-dst softmax and
scatter-add stay local.

Per layer, per core:
 1. Node phase: [h|es|ed] = X @ [W | W@As | W@Ad] (one matmul per 128-node
    tile). ed stays SBUF-resident; [h|es] shards go to DRAM.
 2. AllGather [h|es] shards (in two halves A/B so edge processing of half A
    overlaps the AllGather of half B), then one strided repack DMA into a
    768B-stride gather array HX (row = [h(512B) | es(16B) | pad]).
 3. Edge phase: per-edge rows arrive via dma_gather in slices of <=1024
    indices (the SWDGE descriptor-ring capacity; Q7 descriptor generation at
    ~3ns/row is the bottleneck, which is why h and es ride in ONE stream and
    ed uses no gather at all). Per 128-edge chunk, a one-hot matmul does the
    per-dst-block scatter: psum[d, 0:4] += sum_e onehot[e,d]*w_e (softmax
    denominator) and psum[d, 4:132] += sum_e onehot[e,d]*(w_e*h_e).
    ed[dst_e] is expanded on-chip: OH_de[d,e] (built from a K=1 broadcast
    matmul + is_equal) times the resident ed block. Softmax max-subtraction
    is skipped (logits are ~1e-1, exp is safe; result is mathematically
    identical). Half-A partial sums are evacuated to SBUF so only ~3 PSUM
    banks rotate.
 4. Post per block: normalize, bias, ELU (= relu(x)+exp(min(x,0))-1 with the
    -1 absorbed by LayerNorm's shift invariance), LayerNorm (rsqrt via
    exp(-0.5*ln(v)) so the whole kernel uses one ACT table set), residual
    (layer 1), and a PE transpose to produce the next layer's lhsT.

int16 gather indices limit sources to 32768 rows, so gather arrays are split
into halves by node local index (< 3200 vs >= 3200), each < 32768 rows.
"""

import inspect
import textwrap

import numpy as np

import concourse.bass as bass
import concourse.mybir as mybir
import concourse.tile as tile
from concourse import bacc
from concourse.bass_utils import run_bass_kernel_spmd

# ---- problem constants (hardcoded; must match the grader's reference) ----
N, E = 50000, 800000
NEG = 0.2
EPS = 1e-5
NCORES = 8
NLOC = 6250           # real nodes per core
NSH = 6272            # padded nodes per core (49 tiles of 128)
TILES = NSH // 128    # 49
A_LOC = 3200          # locals < A_LOC -> half A (25 tiles)
B_LOC = NSH - A_LOC   # 3072 (24 tiles)
A_TILES = A_LOC // 128
NA = NCORES * A_LOC   # 25600 (< 32768, int16-safe)
NB = NCORES * B_LOC   # 24576
ROWF = 132            # gathered row: h(128) + es(4) floats
STRIDEF = 192         # gather-array row stride in floats (768B, 256B-aligned)
SLICE_CH = 8          # chunks per gather instruction (1024 idx ring limit)

f32 = mybir.dt.float32
i16 = mybir.dt.int16

TRACE = False
LAST_EXEC_NS = None
LAST_RESULTS = None
_PROGRAM_CACHE = {}


def _patch_dma_gather():
    """Relax dma_gather's elem_size%256 assert (the firmware constraint is on
    the row *stride*, which stays 256B-aligned); enables 528B elements."""
    if getattr(bass.BassGpSimd.dma_gather, "_patched", False):
        return
    src = textwrap.dedent(inspect.getsource(bass.BassGpSimd.dma_gather))
    assert "elem_size_bytes % 256 == 0" in src
    src = src.replace(
        "elem_size_bytes > 0 and elem_size_bytes % 256 == 0",
        "elem_size_bytes > 0",
    )
    ns = vars(bass).copy()
    exec(compile(src, "<patched_dma_gather>", "exec"), ns)
    fn = ns["dma_gather"]
    fn._patched = True
    bass.BassGpSimd.dma_gather = fn


def _install_ntff_hook():
    """Register the axon NTFF profiling hook (antenv.axon_hooks is missing in
    this image) so run_bass_kernel_spmd(trace=True) returns exec_time_ns."""
    import sys
    import types
    if "antenv.axon_hooks" in sys.modules:
        return
    import antenv
    mod = types.ModuleType("antenv.axon_hooks")
    _h = [None]
    mod.set_axon_ntff_profile_hook = lambda h: _h.__setitem__(0, h)
    mod.get_axon_ntff_profile_hook = lambda: _h[0]
    sys.modules["antenv.axon_hooks"] = mod
    antenv.axon_hooks = mod
    from trn_agent_boot.trn_boot import _ntff_profile_via_ctypes
    mod.set_axon_ntff_profile_hook(
        _ntff_profile_via_ctypes("/opt/axon/libaxon_pjrt.so"))


def _slices_of_half(cbh_list, half):
    """Static chunk layout for one half: blocks[j] = dst block of chunk j,
    first/last[j] = whether chunk j is the first/last of its block-half."""
    blocks = []
    first = []
    last = []
    for b in range(TILES):
        n = cbh_list[b][half]
        for j in range(n):
            blocks.append(b)
            first.append(j == 0)
            last.append(j == n - 1)
    return blocks, first, last


def _build_program(cbh_list):
    """Build the SPMD Bass program. cbh_list[b][h] = chunks (128 edges) for
    dst block b, source half h — identical across cores (SPMD)."""
    _patch_dma_gather()
    nc = bacc.Bacc("TRN2", num_swdge_queues=4)
    ch_half = [sum(cbh_list[b][h] for b in range(TILES)) for h in range(2)]

    # ---- external inputs ----
    XT0 = nc.dram_tensor("XT0", [128, NSH], f32, kind="ExternalInput")
    WEXT = nc.dram_tensor("WEXT", [3, 128, 136], f32, kind="ExternalInput")
    WSK = nc.dram_tensor("WSK", [128, 128], f32, kind="ExternalInput")
    BREP = nc.dram_tensor("BREP", [2, 128, 128], f32, kind="ExternalInput")
    GREP = nc.dram_tensor("GREP", [2, 128, 128], f32, kind="ExternalInput")
    BEREP = nc.dram_tensor("BEREP", [2, 128, 128], f32, kind="ExternalInput")
    B3REP = nc.dram_tensor("B3REP", [128, 32], f32, kind="ExternalInput")
    IOTA = nc.dram_tensor("IOTA", [128, 128], f32, kind="ExternalInput")
    IOTAC = nc.dram_tensor("IOTAC", [128, 1], f32, kind="ExternalInput")
    ONESR = nc.dram_tensor("ONESR", [1, 128], f32, kind="ExternalInput")
    IDEN = nc.dram_tensor("IDEN", [128, 128], f32, kind="ExternalInput")
    IDXH = [nc.dram_tensor(f"IDXH{h}", [128, ch_half[h] * 8], i16,
                           kind="ExternalInput") for h in range(2)]
    DOFF = [nc.dram_tensor(f"DOFF{h}", [128, ch_half[h]], f32,
                           kind="ExternalInput") for h in range(2)]
    DOFFR = [nc.dram_tensor(f"DOFFR{h}", [1, ch_half[h] * 128], f32,
                            kind="ExternalInput") for h in range(2)]
    OUT = nc.dram_tensor("OUT", [NSH, 32], f32, kind="ExternalOutput")

    ts = bass.ts
    TT = mybir.AluOpType
    ACT = mybir.ActivationFunctionType

    with tile.TileContext(nc, num_cores=NCORES) as tc:
        with (
            tc.tile_pool(name="dram", bufs=1, space="DRAM") as dp,
            tc.tile_pool(name="consts", bufs=1) as cp,
            tc.tile_pool(name="xtp", bufs=2) as xtp,
            tc.tile_pool(name="node", bufs=3) as npo,
            tc.tile_pool(name="edsb", bufs=2) as edp_,
            tc.tile_pool(name="accsb", bufs=1) as asb,
            tc.tile_pool(name="idx", bufs=3) as ip,
            tc.tile_pool(name="gat", bufs=3) as gp,
            tc.tile_pool(name="ohp", bufs=2) as op_,
            tc.tile_pool(name="edge", bufs=2) as ep,
            tc.tile_pool(name="rhsp", bufs=3) as rp,
            tc.tile_pool(name="post", bufs=2) as qp,
            tc.tile_pool(name="tiny", bufs=4) as tp,
            # PSUM budget (8 banks): acc x3, node/trans/resid x2,
            # ohde x2 (4KB), edpe x1
            tc.tile_pool(name="accps", bufs=3, space="PSUM") as ap_p,
            tc.tile_pool(name="nodeps", bufs=2, space="PSUM") as np_p,
            tc.tile_pool(name="ohdeps", bufs=1, space="PSUM") as oh_p,
            tc.tile_pool(name="edpeps", bufs=1, space="PSUM") as ed_p,
        ):
            # ---- internal DRAM ----
            hx_sh = [dp.tile([A_LOC, ROWF], f32, name="hx_shA"),
                     dp.tile([B_LOC, ROWF], f32, name="hx_shB")]
            # Shared collective outputs must be single-writer: one per layer
            hc_l = [[dp.tile([NA, ROWF], f32, name=f"hc_A{l}",
                             addr_space="Shared"),
                     dp.tile([NB, ROWF], f32, name=f"hc_B{l}",
                             addr_space="Shared")] for l in range(3)]
            hx = [dp.tile([NA, STRIDEF], f32, name="hx_A"),
                  dp.tile([NB, STRIDEF], f32, name="hx_B")]

            # ---- constants ----
            wext_t = cp.tile([128, 3, 136], f32, name="wext_t")
            nc.sync.dma_start(wext_t[:], WEXT[:].rearrange("l p f -> p l f"))
            wsk_t = cp.tile([128, 128], f32, name="wsk_t")
            nc.sync.dma_start(wsk_t[:], WSK[:])
            brep_t = cp.tile([128, 2, 128], f32, name="brep_t")
            nc.sync.dma_start(brep_t[:], BREP[:].rearrange("l p f -> p l f"))
            grep_t = cp.tile([128, 2, 128], f32, name="grep_t")
            nc.sync.dma_start(grep_t[:], GREP[:].rearrange("l p f -> p l f"))
            berep_t = cp.tile([128, 2, 128], f32, name="berep_t")
            nc.sync.dma_start(berep_t[:], BEREP[:].rearrange("l p f -> p l f"))
            b3_t = cp.tile([128, 32], f32, name="b3_t")
            nc.sync.dma_start(b3_t[:], B3REP[:])
            iota_t = cp.tile([128, 128], f32, name="iota_t")
            nc.sync.dma_start(iota_t[:], IOTA[:])
            iotac_t = cp.tile([128, 1], f32, name="iotac_t")
            nc.sync.dma_start(iotac_t[:], IOTAC[:])
            ones_t = cp.tile([1, 128], f32, name="ones_t")
            nc.sync.dma_start(ones_t[:], ONESR[:])
            iden_t = cp.tile([128, 128], f32, name="iden_t")
            nc.sync.dma_start(iden_t[:], IDEN[:])

            xt0_t = xtp.tile([128, NSH], f32, tag="xt", name="xt0_t")
            nc.sync.dma_start(xt0_t[:], XT0[:])

            xt_cur = xt0_t
            for lay in range(3):
                if lay < 2:
                    xt_next = xtp.tile([128, NSH], f32, tag="xt",
                                       name=f"xt{lay + 1}_t")
                # ed values stay on-chip: [128, TILES, 4]
                ed_sb = edp_.tile([128, TILES, 4], f32, tag="ed_sb",
                                  name="ed_sb")

                # ======== NODE PHASE ========
                for t in range(TILES):
                    nps = np_p.tile([128, 136], f32, space="PSUM", tag="nps",
                                    name="nps")
                    nc.tensor.matmul(nps[:], lhsT=xt_cur[:, ts(t, 128)],
                                     rhs=wext_t[:, lay, :], start=True,
                                     stop=True)
                    nsb = npo.tile([128, ROWF], f32, tag="nsb", name="nsb")
                    nc.vector.tensor_copy(nsb[:], nps[:, 0:ROWF])
                    nc.vector.tensor_copy(ed_sb[:, t, :], nps[:, 132:136])
                    if t < A_TILES:
                        nc.sync.dma_start(hx_sh[0][ts(t, 128), :], nsb[:])
                    else:
                        nc.sync.dma_start(hx_sh[1][ts(t - A_TILES, 128), :],
                                          nsb[:])

                # ======== ALLGATHER + REPACK (A then B) ========
                for hf in range(2):
                    nc.gpsimd.collective_compute(
                        "AllGather", mybir.AluOpType.bypass,
                        replica_groups=[list(range(NCORES))],
                        ins=[hx_sh[hf].opt()], outs=[hc_l[lay][hf].opt()])
                    nc.sync.dma_start(hx[hf][:, 0:ROWF], hc_l[lay][hf][:])

                # ======== EDGE PHASE ========
                acc_a = {}   # "tile" -> SBUF tile holding half-A partials
                acc_ps = {}  # block -> live PSUM acc tile

                def emit_slice(hf, s0, blocks, first, last):
                    sc = min(SLICE_CH, len(blocks) - s0)
                    ne = sc * 128
                    idxh_t = ip.tile([128, SLICE_CH * 8], i16, tag="idxh",
                                     name="idxh_t")
                    nc.sync.dma_start(idxh_t[:, 0:sc * 8],
                                      IDXH[hf][:, s0 * 8:(s0 + sc) * 8])
                    doff_t = ip.tile([128, SLICE_CH], f32, tag="doff",
                                     name="doff_t")
                    nc.sync.dma_start(doff_t[:, 0:sc],
                                      DOFF[hf][:, s0:s0 + sc])
                    doffr_t = ip.tile([1, SLICE_CH * 128], f32, tag="doffr",
                                      name="doffr_t")
                    nc.sync.dma_start(doffr_t[:, 0:ne],
                                      DOFFR[hf][:, s0 * 128:s0 * 128 + ne])

                    hg = gp.tile([128, SLICE_CH, ROWF], f32, tag="hg",
                                 name="hg")
                    nc.gpsimd.dma_gather(
                        hg[:, 0:sc, :], hx[hf][:, 0:ROWF],
                        idxh_t[:, 0:sc * 8], ne, ne, ROWF, elem_step=STRIDEF,
                        queue_num=0)

                    # OH_ed[e, c, d] for the scatter matmuls
                    oh = op_.tile([128, SLICE_CH, 128], f32, tag="oh",
                                  name="oh")
                    nc.vector.tensor_tensor(
                        out=oh[:, 0:sc, :],
                        in0=doff_t[:, 0:sc, None].to_broadcast([128, sc, 128]),
                        in1=iota_t[:, None, :].to_broadcast([128, sc, 128]),
                        op=TT.is_equal)

                    # OH_de[d, e] via K=1 broadcast matmul + is_equal
                    ohde_ps = oh_p.tile([128, SLICE_CH * 128], f32,
                                        space="PSUM", tag="ohde",
                                        name="ohde_ps")
                    for q0 in range(0, ne, 512):
                        qn = min(512, ne - q0)
                        nc.tensor.matmul(
                            ohde_ps[:, q0:q0 + qn], lhsT=ones_t[:],
                            rhs=doffr_t[:, q0:q0 + qn], start=True,
                            stop=True, skip_group_check=True)
                    ohde = ep.tile([128, SLICE_CH * 128], f32, tag="ohde",
                                   name="ohde")
                    nc.vector.tensor_tensor(
                        out=ohde[:, 0:ne], in0=ohde_ps[:, 0:ne],
                        in1=iotac_t[:, :].to_broadcast([128, ne]),
                        op=TT.is_equal)

                    # ed per edge: edpe[:, c, :] = OH_de_c.T @ ed_blk
                    edpe = ed_p.tile([128, SLICE_CH, 4], f32, space="PSUM",
                                     tag="edpe", name="edpe")
                    for c in range(sc):
                        nc.tensor.matmul(
                            edpe[:, c, :],
                            lhsT=ohde[:, (c * 128):(c * 128 + 128)],
                            rhs=ed_sb[:, blocks[s0 + c], :],
                            start=True, stop=True, skip_group_check=True)

                    # w = exp(leakyrelu(es + ed))
                    z = ep.tile([128, SLICE_CH, 4], f32, tag="z", name="z")
                    nc.vector.tensor_tensor(
                        out=z[:, 0:sc, :], in0=hg[:, 0:sc, 128:132],
                        in1=edpe[:, 0:sc, :], op=TT.add)
                    zl = ep.tile([128, SLICE_CH, 4], f32, tag="zl", name="zl")
                    nc.vector.tensor_scalar(
                        out=zl[:, 0:sc, :], in0=z[:, 0:sc, :], scalar1=NEG,
                        scalar2=None, op0=TT.mult)
                    lr = ep.tile([128, SLICE_CH, 4], f32, tag="lr", name="lr")
                    nc.vector.tensor_tensor(
                        out=lr[:, 0:sc, :], in0=z[:, 0:sc, :],
                        in1=zl[:, 0:sc, :], op=TT.max)
                    rhs_t = rp.tile([128, SLICE_CH, ROWF], f32, tag="rhs",
                                    name="rhs_t")
                    nc.scalar.activation(rhs_t[:, 0:sc, 0:4], lr[:, 0:sc, :],
                                         ACT.Exp)
                    nc.vector.tensor_tensor(
                        out=rhs_t[:, 0:sc, 4:132].rearrange(
                            "p c (h v) -> p c h v", h=4),
                        in0=hg[:, 0:sc, 0:128].rearrange(
                            "p c (h v) -> p c h v", h=4),
                        in1=rhs_t[:, 0:sc, 0:4][:, :, :, None].to_broadcast(
                            [128, sc, 4, 32]),
                        op=TT.mult)
                    return oh, rhs_t, sc

                def emit_post(b, tot):
                    dn = tp.tile([128, 4], f32, tag="dn", name="dn")
                    nc.vector.tensor_scalar(
                        out=dn[:], in0=tot[:, 0:4], scalar1=1e-16,
                        scalar2=None, op0=TT.add)
                    rc = tp.tile([128, 4], f32, tag="rc", name="rc")
                    nc.vector.reciprocal(rc[:], dn[:])
                    onorm = qp.tile([128, 128], f32, tag="onorm",
                                    name="onorm")
                    nc.vector.tensor_tensor(
                        out=onorm[:].rearrange("p (h v) -> p h v", h=4),
                        in0=tot[:, 4:132].rearrange("p (h v) -> p h v", h=4),
                        in1=rc[:, :, None].to_broadcast([128, 4, 32]),
                        op=TT.mult)
                    if lay == 2:
                        hm = qp.tile([128, 32], f32, tag="hm", name="hm")
                        nc.vector.tensor_reduce(
                            out=hm[:],
                            in_=onorm[:].rearrange("p (h v) -> p v h", h=4),
                            axis=mybir.AxisListType.X, op=TT.add)
                        hb = qp.tile([128, 32], f32, tag="hb", name="hb")
                        nc.vector.tensor_scalar(
                            out=hb[:], in0=hm[:], scalar1=0.25, scalar2=None,
                            op0=TT.mult)
                        ho = qp.tile([128, 32], f32, tag="ho", name="ho")
                        nc.vector.tensor_tensor(out=ho[:], in0=hb[:],
                                                in1=b3_t[:], op=TT.add)
                        nc.sync.dma_start(OUT[ts(b, 128), :], ho[:])
                        return
                    u0 = qp.tile([128, 128], f32, tag="u0", name="u0")
                    nc.vector.tensor_tensor(out=u0[:], in0=onorm[:],
                                            in1=brep_t[:, lay, :], op=TT.add)
                    mm_ = qp.tile([128, 128], f32, tag="mm_", name="mm_")
                    nc.vector.tensor_scalar(out=mm_[:], in0=u0[:], scalar1=0.0,
                                            scalar2=None, op0=TT.min)
                    em = qp.tile([128, 128], f32, tag="em", name="em")
                    nc.scalar.activation(em[:], mm_[:], ACT.Exp)
                    rl = qp.tile([128, 128], f32, tag="rl", name="rl")
                    nc.vector.tensor_scalar(out=rl[:], in0=u0[:], scalar1=0.0,
                                            scalar2=None, op0=TT.max)
                    u = qp.tile([128, 128], f32, tag="u", name="u")
                    nc.vector.tensor_tensor(out=u[:], in0=em[:], in1=rl[:],
                                            op=TT.add)
                    ss = tp.tile([128, 1], f32, tag="ss", name="ss")
                    nc.vector.tensor_reduce(out=ss[:], in_=u[:],
                                            axis=mybir.AxisListType.X,
                                            op=TT.add)
                    mu = tp.tile([128, 1], f32, tag="mu", name="mu")
                    nc.vector.tensor_scalar(out=mu[:], in0=ss[:],
                                            scalar1=1.0 / 128, scalar2=None,
                                            op0=TT.mult)
                    xc = qp.tile([128, 128], f32, tag="xc", name="xc")
                    nc.vector.tensor_scalar(out=xc[:], in0=u[:], scalar1=mu[:],
                                            scalar2=None, op0=TT.subtract)
                    sqd = qp.tile([128, 128], f32, tag="sqd", name="sqd")
                    ssq = tp.tile([128, 1], f32, tag="ssq", name="ssq")
                    nc.scalar.activation(sqd[:], xc[:], ACT.Square,
                                         accum_out=ssq[:])
                    var = tp.tile([128, 1], f32, tag="var", name="var")
                    nc.vector.tensor_scalar(out=var[:], in0=ssq[:],
                                            scalar1=1.0 / 128, scalar2=EPS,
                                            op0=TT.mult, op1=TT.add)
                    lnv = tp.tile([128, 1], f32, tag="lnv", name="lnv")
                    nc.scalar.activation(lnv[:], var[:], ACT.Ln)
                    rstd = tp.tile([128, 1], f32, tag="rstd", name="rstd")
                    nc.scalar.activation(rstd[:], lnv[:], ACT.Exp, scale=-0.5)
                    xn = qp.tile([128, 128], f32, tag="xn", name="xn")
                    nc.vector.tensor_scalar(out=xn[:], in0=xc[:],
                                            scalar1=rstd[:], scalar2=None,
                                            op0=TT.mult)
                    xg = qp.tile([128, 128], f32, tag="xg", name="xg")
                    nc.vector.tensor_tensor(out=xg[:], in0=xn[:],
                                            in1=grep_t[:, lay, :], op=TT.mult)
                    xb = qp.tile([128, 128], f32, tag="xb", name="xb")
                    nc.vector.tensor_tensor(out=xb[:], in0=xg[:],
                                            in1=berep_t[:, lay, :], op=TT.add)
                    if lay == 0:
                        rps = np_p.tile([128, 128], f32, space="PSUM",
                                        tag="nps", name="rps")
                        nc.tensor.matmul(rps[:], lhsT=xt0_t[:, ts(b, 128)],
                                         rhs=wsk_t[:], start=True, stop=True)
                        xf = qp.tile([128, 128], f32, tag="xf", name="xf")
                        nc.vector.tensor_tensor(out=xf[:], in0=xb[:],
                                                in1=rps[:], op=TT.add)
                    else:
                        xf = xb
                    tps = np_p.tile([128, 128], f32, space="PSUM", tag="nps",
                                    name="tps")
                    nc.tensor.transpose(tps[:], xf[:], iden_t[:])
                    nc.vector.tensor_copy(xt_next[:, ts(b, 128)], tps[:])

                def emit_chunks(hf, s0, blocks, first, last, oh, rhs_t,
                                sc):
                    for c in range(sc):
                        j = s0 + c
                        b = blocks[j]
                        if first[j]:
                            acc_ps[b] = ap_p.tile(
                                [128, ROWF], f32, space="PSUM", tag="acc",
                                name="acc")
                        nc.tensor.matmul(
                            acc_ps[b][:], lhsT=oh[:, c, :],
                            rhs=rhs_t[:, c, :], start=first[j],
                            stop=last[j], skip_group_check=True)
                        if not last[j]:
                            continue
                        if hf == 0:
                            if "tile" not in acc_a:
                                acc_a["tile"] = asb.tile(
                                    [128, TILES, ROWF], f32, tag="acc_a",
                                    name="acc_a")
                            nc.vector.tensor_copy(
                                acc_a["tile"][:, b, :], acc_ps[b][:])
                            del acc_ps[b]
                            continue
                        tot = qp.tile([128, ROWF], f32, tag="tot", name="tot")
                        nc.vector.tensor_tensor(
                            out=tot[:], in0=acc_a["tile"][:, b, :],
                            in1=acc_ps[b][:], op=TT.add)
                        del acc_ps[b]
                        emit_post(b, tot)

                for hf in range(2):
                    blocks, first, last = _slices_of_half(cbh_list, hf)
                    for s0 in range(0, len(blocks), SLICE_CH):
                        oh, rhs_t, sc = emit_slice(hf, s0, blocks, first,
                                                   last)
                        emit_chunks(hf, s0, blocks, first, last, oh, rhs_t,
                                    sc)
                if lay < 2:
                    xt_cur = xt_next

    # Align gather queue_num with Tile's round-robin DMASW lane assignment
    # (lane i%8 <-> queue i%4 in scheduled Pool order) so each semaphore
    # lane is only ever used by a single SWDGE queue.
    gi = 0
    for bb in nc.main_func.blocks:
        for ins in bb.instructions:
            if isinstance(ins, mybir.InstDMAGatherAnt):
                ins.queue_num = gi % 4
                gi += 1
    nc.compile()
    return nc


def _wrap16(v):
    n = len(v)
    w = v.reshape(n // 16, 16).T  # [16, n/16]
    return np.tile(w, (8, 1)).astype(np.int16)


def _block_diag_att(a):
    # a: [4, C] -> [4*C, 4] block diagonal
    c = a.shape[1]
    out = np.zeros((4 * c, 4), np.float32)
    for h in range(4):
        out[h * c:(h + 1) * c, h] = a[h]
    return out


def prepare_inputs(x, edge_index, W1, as1, ad1, b1, g1, be1, W2, as2, ad2,
                   b2, g2, be2, W3, as3, ad3, b3, Wsk, bsk):
    x = np.asarray(x, np.float32)
    ei = np.asarray(edge_index)
    src = np.concatenate([ei[0], np.arange(N)]).astype(np.int64)
    dst = np.concatenate([ei[1], np.arange(N)]).astype(np.int64)

    score, sloc = src // NLOC, src % NLOC
    half = (sloc >= A_LOC).astype(np.int64)
    gidx = np.where(half == 0, A_LOC * score + sloc,
                    B_LOC * score + (sloc - A_LOC))
    dcore, dloc = dst // NLOC, dst % NLOC
    blk = dloc >> 7
    doff = dloc & 127

    # per-(block, half) chunk counts: max over cores (SPMD-identical program)
    gid = (dcore * TILES + blk) * 2 + half
    counts = np.bincount(gid, minlength=NCORES * TILES * 2).reshape(
        NCORES, TILES, 2)
    cbh = np.maximum(1, -(-counts.max(axis=0) // 128))  # [TILES, 2]
    cbh_list = tuple(tuple(int(v) for v in row) for row in cbh)

    # chunk offset of each (block, half) within its half's chunk sequence
    off = np.zeros((TILES, 2), np.int64)
    for h in range(2):
        off[:, h] = np.concatenate([[0], np.cumsum(cbh[:, h])[:-1]])
    ch_half = [int(cbh[:, h].sum()) for h in range(2)]

    in_maps = []
    wext = np.stack([
        np.concatenate([
            np.asarray(W, np.float32),
            np.asarray(W, np.float32) @ _block_diag_att(
                np.asarray(a_s, np.float32)),
            np.asarray(W, np.float32) @ _block_diag_att(
                np.asarray(a_d, np.float32))], axis=1)
        for (W, a_s, a_d) in [(W1, as1, ad1), (W2, as2, ad2), (W3, as3, ad3)]
    ]).astype(np.float32)
    brep = np.stack([np.broadcast_to(np.asarray(b1, np.float32), (128, 128)),
                     np.broadcast_to(np.asarray(b2, np.float32), (128, 128))])
    grep = np.stack([np.broadcast_to(np.asarray(g1, np.float32), (128, 128)),
                     np.broadcast_to(np.asarray(g2, np.float32), (128, 128))])
    berep = np.stack([
        np.broadcast_to(np.asarray(be1, np.float32)
                        + np.asarray(bsk, np.float32), (128, 128)),
        np.broadcast_to(np.asarray(be2, np.float32), (128, 128))])
    b3rep = np.ascontiguousarray(
        np.broadcast_to(np.asarray(b3, np.float32), (128, 32)))
    iota = np.tile(np.arange(128, dtype=np.float32), (128, 1))
    iotac = np.arange(128, dtype=np.float32).reshape(128, 1)
    onesr = np.ones((1, 128), np.float32)
    iden = np.eye(128, dtype=np.float32)
    wsk = np.asarray(Wsk, np.float32)

    for c in range(NCORES):
        m = dcore == c
        e_blk, e_half, e_gidx, e_doff = blk[m], half[m], gidx[m], doff[m]
        core_map = {}
        for hf in range(2):
            mh = e_half == hf
            b_, g_, d_ = e_blk[mh], e_gidx[mh], e_doff[mh]
            order = np.lexsort((g_, b_))
            b_, g_, d_ = b_[order], g_[order], d_[order]
            starts = np.searchsorted(b_, np.arange(TILES))
            rank = np.arange(len(b_)) - starts[b_]
            pos = off[b_, hf] * 128 + rank
            ne = ch_half[hf] * 128
            gi = np.zeros(ne, np.int64)
            gd = np.full(ne, -1.0, np.float64)
            gi[pos] = g_
            gd[pos] = d_
            core_map[f"IDXH{hf}"] = _wrap16(gi)
            core_map[f"DOFF{hf}"] = np.ascontiguousarray(
                gd.reshape(-1, 128).T.astype(np.float32))
            core_map[f"DOFFR{hf}"] = gd.reshape(1, -1).astype(np.float32)
        xt = np.zeros((128, NSH), np.float32)
        xt[:, :NLOC] = x[c * NLOC:(c + 1) * NLOC].T
        core_map.update({
            "XT0": xt, "WEXT": wext, "WSK": wsk, "BREP": brep, "GREP": grep,
            "BEREP": berep, "B3REP": b3rep, "IOTA": iota, "IOTAC": iotac,
            "ONESR": onesr, "IDEN": iden,
        })
        in_maps.append(core_map)

    return in_maps, cbh_list


def kernel(**inputs):
    global LAST_EXEC_NS, LAST_RESULTS
    in_maps, cbh_list = prepare_inputs(**inputs)
    if cbh_list not in _PROGRAM_CACHE:
        _PROGRAM_CACHE[cbh_list] = _build_program(cbh_list)
    nc = _PROGRAM_CACHE[cbh_list]

    if TRACE:
        _install_ntff_hook()
    res = run_bass_kernel_spmd(nc, in_maps, list(range(NCORES)),
                               trace=TRACE)
    LAST_EXEC_NS = res.exec_time_ns
    LAST_RESULTS = res
    out = np.concatenate(
        [res.results[c]["OUT"][:NLOC] for c in range(NCORES)], axis=0)
    return out.astype(np.float32)



# revision 4
# speedup vs baseline: 1.3547x; 1.3547x over previous
"""3-layer GAT on 8 Trainium2 NeuronCores (Bass/Tile SPMD), v2 (bf16).

Sharding: nodes partitioned into 8 contiguous blocks of 6250 (padded to 6272);
edges assigned to the core owning their dst node, so per-dst softmax and
scatter-add stay local.

Per layer, per core:
 1. Node phase: [h|es|ed] = X @ [W | W@As | W@Ad] (one bf16 matmul per
    128-node tile). [h|es] rows go to DRAM as bf16 in a 512B-stride
    gather-ready layout; ed rows go to a local 256B-stride edx table.
 2. AllGather the [h|es] shards directly in the strided layout (no repack),
    in two halves A/B so edge processing of half A overlaps the AllGather
    of half B.
 3. Edge phase per 1024-edge slice: one dma_gather pulls [h|es] rows
    (264B payload each) from the gathered array; a second tiny dma_gather
    pulls ed[dst] (8B each) from edx. w = exp(leakyrelu(es+ed)) and
    msg = h*w are computed in bf16; a one-hot matmul per 128-edge chunk
    scatter-adds [w | msg] into the dst block's PSUM accumulator.
    Softmax max-subtraction is skipped (logits are ~1e-1; mathematically
    identical result).
 4. Post per block: normalize, bias, ELU (= relu(x)+exp(min(x,0))-1 with
    the -1 absorbed by LayerNorm's shift invariance), LayerNorm (rsqrt via
    exp(-0.5*ln(v))), residual (layer 1), and a PE transpose to produce the
    next layer's bf16 lhsT. All ACT functions (exp/ln/square) live in the
    single pinned table set natural_log_exp_and_others, so the table loads
    once.

int16 gather indices limit sources to 32768 rows, so gather arrays are split
into halves by node local index (< 3200 vs >= 3200), each < 32768 rows.
"""

import inspect
import textwrap

import numpy as np
import ml_dtypes

import concourse.bass as bass
import concourse.mybir as mybir
import concourse.tile as tile
from concourse import bacc
from concourse.bass_utils import run_bass_kernel_spmd

# ---- problem constants (hardcoded; must match the grader's reference) ----
N, E = 50000, 800000
NEG = 0.2
EPS = 1e-5
NCORES = 8
NLOC = 6250           # real nodes per core
NSH = 6272            # padded nodes per core (49 tiles of 128)
TILES = NSH // 128    # 49
A_LOC = 3200          # locals < A_LOC -> half A (25 tiles)
B_LOC = NSH - A_LOC   # 3072 (24 tiles)
A_TILES = A_LOC // 128
NA = NCORES * A_LOC   # 25600 (< 32768, int16-safe)
NB = NCORES * B_LOC   # 24576
ROWF = 132            # gathered row: h(128) + es(4) bf16 values
STRIDEE = 256         # gather-array row stride in bf16 elems (512B)
SLICE_CH = 8          # chunks per gather instruction (1024 idx ring limit)

f32 = mybir.dt.float32
bf16 = mybir.dt.bfloat16
i16 = mybir.dt.int16

TRACE = False
LAST_EXEC_NS = None
LAST_RESULTS = None
_PROGRAM_CACHE = {}


def _patch_dma_gather():
    """Relax dma_gather's elem_size%256 assert (the firmware constraint is on
    the row *stride*, which stays 256B-aligned); enables 264B/8B elements."""
    if getattr(bass.BassGpSimd.dma_gather, "_patched", False):
        return
    src = textwrap.dedent(inspect.getsource(bass.BassGpSimd.dma_gather))
    assert "elem_size_bytes % 256 == 0" in src
    src = src.replace(
        "elem_size_bytes > 0 and elem_size_bytes % 256 == 0",
        "elem_size_bytes > 0",
    )
    ns = vars(bass).copy()
    exec(compile(src, "<patched_dma_gather>", "exec"), ns)
    fn = ns["dma_gather"]
    fn._patched = True
    bass.BassGpSimd.dma_gather = fn


def _patch_act_tables():
    """Confine exp/ln/square to the one set that has all three
    (natural_log_exp_and_others) so the ACT table loads exactly once,
    instead of thrashing Exp<->Ln every LayerNorm."""
    if getattr(bacc, "_act_tables_pinned", False):
        return
    orig = bacc.get_activation_tables

    def pinned(arch):
        tabs = dict(orig(arch))  # preserves insertion order == set ids
        tgt = "natural_log_exp_and_others"
        if tgt in tabs:
            drop = {
                mybir.ActivationFunctionType.Exp,
                mybir.ActivationFunctionType.Ln,
                mybir.ActivationFunctionType.Square,
            }
            tabs = {
                name: (s if name == tgt else s - drop)
                for name, s in tabs.items()
            }
        return tabs

    bacc.get_activation_tables = pinned
    bacc._act_tables_pinned = True


def _install_ntff_hook():
    """Register the axon NTFF profiling hook (antenv.axon_hooks is missing in
    this image) so run_bass_kernel_spmd(trace=True) returns exec_time_ns."""
    import sys
    import types
    if "antenv.axon_hooks" in sys.modules:
        return
    import antenv
    mod = types.ModuleType("antenv.axon_hooks")
    _h = [None]
    mod.set_axon_ntff_profile_hook = lambda h: _h.__setitem__(0, h)
    mod.get_axon_ntff_profile_hook = lambda: _h[0]
    sys.modules["antenv.axon_hooks"] = mod
    antenv.axon_hooks = mod
    from trn_agent_boot.trn_boot import _ntff_profile_via_ctypes
    mod.set_axon_ntff_profile_hook(
        _ntff_profile_via_ctypes("/opt/axon/libaxon_pjrt.so"))


def _slices_of_half(cbh_list, half):
    """Static chunk layout for one half: blocks[j] = dst block of chunk j,
    first/last[j] = whether chunk j is the first/last of its block-half."""
    blocks = []
    first = []
    last = []
    for b in range(TILES):
        n = cbh_list[b][half]
        for j in range(n):
            blocks.append(b)
            first.append(j == 0)
            last.append(j == n - 1)
    return blocks, first, last


def _build_program(cbh_list):
    """Build the SPMD Bass program. cbh_list[b][h] = chunks (128 edges) for
    dst block b, source half h — identical across cores (SPMD)."""
    _patch_dma_gather()
    _patch_act_tables()
    nc = bacc.Bacc("TRN2", num_swdge_queues=4)
    ch_half = [sum(cbh_list[b][h] for b in range(TILES)) for h in range(2)]

    # ---- external inputs ----
    XT0 = nc.dram_tensor("XT0", [128, NSH], bf16, kind="ExternalInput")
    WEXT = nc.dram_tensor("WEXT", [3, 128, 136], bf16, kind="ExternalInput")
    WSK = nc.dram_tensor("WSK", [128, 128], bf16, kind="ExternalInput")
    BREP = nc.dram_tensor("BREP", [2, 128, 128], f32, kind="ExternalInput")
    GREP = nc.dram_tensor("GREP", [2, 128, 128], f32, kind="ExternalInput")
    BEREP = nc.dram_tensor("BEREP", [2, 128, 128], f32, kind="ExternalInput")
    B3REP = nc.dram_tensor("B3REP", [128, 32], f32, kind="ExternalInput")
    IOTA = nc.dram_tensor("IOTA", [128, 128], bf16, kind="ExternalInput")
    IDEN = nc.dram_tensor("IDEN", [128, 128], bf16, kind="ExternalInput")
    # per half: [128, 2, ch*8] int16 — [:,0,:] = src gather idx, [:,1,:] = dst
    IDX2 = [nc.dram_tensor(f"IDX2{h}", [128, 2, ch_half[h] * 8], i16,
                           kind="ExternalInput") for h in range(2)]
    DOFF = [nc.dram_tensor(f"DOFF{h}", [128, ch_half[h]], bf16,
                           kind="ExternalInput") for h in range(2)]
    OUT = nc.dram_tensor("OUT", [NSH, 32], f32, kind="ExternalOutput")

    ts = bass.ts
    TT = mybir.AluOpType
    ACT = mybir.ActivationFunctionType

    with tile.TileContext(nc, num_cores=NCORES) as tc:
        with (
            tc.tile_pool(name="dram", bufs=1, space="DRAM") as dp,
            tc.tile_pool(name="consts", bufs=1) as cp,
            tc.tile_pool(name="xtp", bufs=2) as xtp,
            tc.tile_pool(name="node", bufs=3) as npo,
            tc.tile_pool(name="edsb", bufs=2) as edp_,
            tc.tile_pool(name="accsb", bufs=1) as asb,
            tc.tile_pool(name="idx", bufs=3) as ip,
            tc.tile_pool(name="gat", bufs=3) as gp,
            tc.tile_pool(name="ohp", bufs=2) as op_,
            tc.tile_pool(name="edge", bufs=2) as ep,
            tc.tile_pool(name="rhsp", bufs=3) as rp,
            tc.tile_pool(name="post", bufs=2) as qp,
            tc.tile_pool(name="tiny", bufs=4) as tp,
            # PSUM budget (8 banks): acc x4, node/trans/resid x2
            tc.tile_pool(name="accps", bufs=4, space="PSUM") as ap_p,
            tc.tile_pool(name="nodeps", bufs=2, space="PSUM") as np_p,
        ):
            # ---- internal DRAM ----
            hx_sh = [dp.tile([A_LOC, STRIDEE], bf16, name="hx_shA"),
                     dp.tile([B_LOC, STRIDEE], bf16, name="hx_shB")]
            # Shared collective outputs must be single-writer: one per layer
            hc_l = [[dp.tile([NA, STRIDEE], bf16, name=f"hc_A{l}",
                             addr_space="Shared"),
                     dp.tile([NB, STRIDEE], bf16, name=f"hc_B{l}",
                             addr_space="Shared")] for l in range(3)]
            # local ed table: row per node, 256B stride, first 4 cols used
            edx = dp.tile([NSH, 128], bf16, name="edx")

            # ---- constants ----
            wext_t = cp.tile([128, 3, 136], bf16, name="wext_t")
            nc.sync.dma_start(wext_t[:], WEXT[:].rearrange("l p f -> p l f"))
            wsk_t = cp.tile([128, 128], bf16, name="wsk_t")
            nc.sync.dma_start(wsk_t[:], WSK[:])
            brep_t = cp.tile([128, 2, 128], f32, name="brep_t")
            nc.sync.dma_start(brep_t[:], BREP[:].rearrange("l p f -> p l f"))
            grep_t = cp.tile([128, 2, 128], f32, name="grep_t")
            nc.sync.dma_start(grep_t[:], GREP[:].rearrange("l p f -> p l f"))
            berep_t = cp.tile([128, 2, 128], f32, name="berep_t")
            nc.sync.dma_start(berep_t[:], BEREP[:].rearrange("l p f -> p l f"))
            b3_t = cp.tile([128, 32], f32, name="b3_t")
            nc.sync.dma_start(b3_t[:], B3REP[:])
            iota_t = cp.tile([128, 128], bf16, name="iota_t")
            nc.sync.dma_start(iota_t[:], IOTA[:])
            iden_t = cp.tile([128, 128], bf16, name="iden_t")
            nc.sync.dma_start(iden_t[:], IDEN[:])

            xt0_t = xtp.tile([128, NSH], bf16, tag="xt", name="xt0_t")
            nc.sync.dma_start(xt0_t[:], XT0[:])

            xt_cur = xt0_t
            for lay in range(3):
                if lay < 2:
                    xt_next = xtp.tile([128, NSH], bf16, tag="xt",
                                       name=f"xt{lay + 1}_t")
                # ed values: staged in SBUF then stored to edx (DRAM)
                eds = edp_.tile([128, TILES, 4], bf16, tag="eds", name="eds")

                # ======== NODE PHASE ========
                for t in range(TILES):
                    nps = np_p.tile([128, 136], f32, space="PSUM", tag="nps",
                                    name="nps")
                    nc.tensor.matmul(nps[:], lhsT=xt_cur[:, ts(t, 128)],
                                     rhs=wext_t[:, lay, :], start=True,
                                     stop=True)
                    nsb = npo.tile([128, ROWF], bf16, tag="nsb", name="nsb")
                    nc.vector.tensor_copy(nsb[:], nps[:, 0:ROWF])
                    nc.vector.tensor_copy(eds[:, t, :], nps[:, 132:136])
                    if t < A_TILES:
                        nc.sync.dma_start(
                            hx_sh[0][ts(t, 128), 0:ROWF], nsb[:])
                    else:
                        nc.sync.dma_start(
                            hx_sh[1][ts(t - A_TILES, 128), 0:ROWF], nsb[:])
                # store ed table: edx[(t*128+p), 0:4] = eds[p, t, :]
                nc.sync.dma_start(
                    edx[:, 0:4].rearrange("(t p) f -> p t f", p=128),
                    eds[:])

                # ======== ALLGATHER (A then B), gather-ready layout ========
                for hf in range(2):
                    nc.gpsimd.collective_compute(
                        "AllGather", mybir.AluOpType.bypass,
                        replica_groups=[list(range(NCORES))],
                        ins=[hx_sh[hf].opt()], outs=[hc_l[lay][hf].opt()])

                # ======== EDGE PHASE ========
                acc_a = {}   # "tile" -> SBUF tile holding half-A partials
                acc_ps = {}  # block -> live PSUM acc tile

                def emit_slice(hf, s0, blocks, first, last):
                    sc = min(SLICE_CH, len(blocks) - s0)
                    ne = sc * 128
                    idx2_t = ip.tile([128, 2, SLICE_CH * 8], i16, tag="idx2",
                                     name="idx2_t")
                    nc.sync.dma_start(idx2_t[:, :, 0:sc * 8],
                                      IDX2[hf][:, :, s0 * 8:(s0 + sc) * 8])
                    doff_t = ip.tile([128, SLICE_CH], bf16, tag="doff",
                                     name="doff_t")
                    nc.sync.dma_start(doff_t[:, 0:sc],
                                      DOFF[hf][:, s0:s0 + sc])

                    hg = gp.tile([128, SLICE_CH, ROWF], bf16, tag="hg",
                                 name="hg")
                    nc.gpsimd.dma_gather(
                        hg[:, 0:sc, :], hc_l[lay][hf][:, 0:ROWF],
                        idx2_t[:, 0, 0:sc * 8], ne, ne, ROWF,
                        elem_step=STRIDEE, queue_num=0)
                    edg = gp.tile([128, SLICE_CH, 4], bf16, tag="edg",
                                  name="edg")
                    nc.gpsimd.dma_gather(
                        edg[:, 0:sc, :], edx[:, 0:4],
                        idx2_t[:, 1, 0:sc * 8], ne, ne, 4,
                        elem_step=128, queue_num=0)

                    # OH[e, c, d] one-hot for the scatter matmuls
                    oh = op_.tile([128, SLICE_CH, 128], bf16, tag="oh",
                                  name="oh")
                    nc.vector.tensor_tensor(
                        out=oh[:, 0:sc, :],
                        in0=doff_t[:, 0:sc, None].to_broadcast([128, sc, 128]),
                        in1=iota_t[:, None, :].to_broadcast([128, sc, 128]),
                        op=TT.is_equal)

                    # w = exp(leakyrelu(es + ed))
                    z = ep.tile([128, SLICE_CH, 4], bf16, tag="z", name="z")
                    nc.vector.tensor_tensor(
                        out=z[:, 0:sc, :], in0=hg[:, 0:sc, 128:132],
                        in1=edg[:, 0:sc, :], op=TT.add)
                    zl = ep.tile([128, SLICE_CH, 4], bf16, tag="zl",
                                 name="zl")
                    nc.vector.tensor_scalar(
                        out=zl[:, 0:sc, :], in0=z[:, 0:sc, :], scalar1=NEG,
                        scalar2=None, op0=TT.mult)
                    lr = ep.tile([128, SLICE_CH, 4], bf16, tag="lr",
                                 name="lr")
                    nc.vector.tensor_tensor(
                        out=lr[:, 0:sc, :], in0=z[:, 0:sc, :],
                        in1=zl[:, 0:sc, :], op=TT.max)
                    rhs_t = rp.tile([128, SLICE_CH, ROWF], bf16, tag="rhs",
                                    name="rhs_t")
                    nc.scalar.activation(rhs_t[:, 0:sc, 0:4], lr[:, 0:sc, :],
                                         ACT.Exp)
                    nc.vector.tensor_tensor(
                        out=rhs_t[:, 0:sc, 4:132].rearrange(
                            "p c (h v) -> p c h v", h=4),
                        in0=hg[:, 0:sc, 0:128].rearrange(
                            "p c (h v) -> p c h v", h=4),
                        in1=rhs_t[:, 0:sc, 0:4][:, :, :, None].to_broadcast(
                            [128, sc, 4, 32]),
                        op=TT.mult)
                    return oh, rhs_t, sc

                def emit_post(b, tot):
                    dn = tp.tile([128, 4], f32, tag="dn", name="dn")
                    nc.vector.tensor_scalar(
                        out=dn[:], in0=tot[:, 0:4], scalar1=1e-16,
                        scalar2=None, op0=TT.add)
                    rc = tp.tile([128, 4], f32, tag="rc", name="rc")
                    nc.vector.reciprocal(rc[:], dn[:])
                    onorm = qp.tile([128, 128], f32, tag="onorm",
                                    name="onorm")
                    nc.vector.tensor_tensor(
                        out=onorm[:].rearrange("p (h v) -> p h v", h=4),
                        in0=tot[:, 4:132].rearrange("p (h v) -> p h v", h=4),
                        in1=rc[:, :, None].to_broadcast([128, 4, 32]),
                        op=TT.mult)
                    if lay == 2:
                        hm = qp.tile([128, 32], f32, tag="hm", name="hm")
                        nc.vector.tensor_reduce(
                            out=hm[:],
                            in_=onorm[:].rearrange("p (h v) -> p v h", h=4),
                            axis=mybir.AxisListType.X, op=TT.add)
                        hb = qp.tile([128, 32], f32, tag="hb", name="hb")
                        nc.vector.tensor_scalar(
                            out=hb[:], in0=hm[:], scalar1=0.25, scalar2=None,
                            op0=TT.mult)
                        ho = qp.tile([128, 32], f32, tag="ho", name="ho")
                        nc.vector.tensor_tensor(out=ho[:], in0=hb[:],
                                                in1=b3_t[:], op=TT.add)
                        nc.sync.dma_start(OUT[ts(b, 128), :], ho[:])
                        return
                    u0 = qp.tile([128, 128], f32, tag="u0", name="u0")
                    nc.vector.tensor_tensor(out=u0[:], in0=onorm[:],
                                            in1=brep_t[:, lay, :], op=TT.add)
                    mm_ = qp.tile([128, 128], f32, tag="mm_", name="mm_")
                    nc.vector.tensor_scalar(out=mm_[:], in0=u0[:], scalar1=0.0,
                                            scalar2=None, op0=TT.min)
                    em = qp.tile([128, 128], f32, tag="em", name="em")
                    nc.scalar.activation(em[:], mm_[:], ACT.Exp)
                    rl = qp.tile([128, 128], f32, tag="rl", name="rl")
                    nc.vector.tensor_scalar(out=rl[:], in0=u0[:], scalar1=0.0,
                                            scalar2=None, op0=TT.max)
                    u = qp.tile([128, 128], f32, tag="u", name="u")
                    nc.vector.tensor_tensor(out=u[:], in0=em[:], in1=rl[:],
                                            op=TT.add)
                    ss = tp.tile([128, 1], f32, tag="ss", name="ss")
                    nc.vector.tensor_reduce(out=ss[:], in_=u[:],
                                            axis=mybir.AxisListType.X,
                                            op=TT.add)
                    mu = tp.tile([128, 1], f32, tag="mu", name="mu")
                    nc.vector.tensor_scalar(out=mu[:], in0=ss[:],
                                            scalar1=1.0 / 128, scalar2=None,
                                            op0=TT.mult)
                    xc = qp.tile([128, 128], f32, tag="xc", name="xc")
                    nc.vector.tensor_scalar(out=xc[:], in0=u[:], scalar1=mu[:],
                                            scalar2=None, op0=TT.subtract)
                    sqd = qp.tile([128, 128], f32, tag="sqd", name="sqd")
                    ssq = tp.tile([128, 1], f32, tag="ssq", name="ssq")
                    nc.scalar.activation(sqd[:], xc[:], ACT.Square,
                                         accum_out=ssq[:])
                    var = tp.tile([128, 1], f32, tag="var", name="var")
                    nc.vector.tensor_scalar(out=var[:], in0=ssq[:],
                                            scalar1=1.0 / 128, scalar2=EPS,
                                            op0=TT.mult, op1=TT.add)
                    lnv = tp.tile([128, 1], f32, tag="lnv", name="lnv")
                    nc.scalar.activation(lnv[:], var[:], ACT.Ln)
                    rstd = tp.tile([128, 1], f32, tag="rstd", name="rstd")
                    nc.scalar.activation(rstd[:], lnv[:], ACT.Exp, scale=-0.5)
                    xn = qp.tile([128, 128], f32, tag="xn", name="xn")
                    nc.vector.tensor_scalar(out=xn[:], in0=xc[:],
                                            scalar1=rstd[:], scalar2=None,
                                            op0=TT.mult)
                    xg = qp.tile([128, 128], f32, tag="xg", name="xg")
                    nc.vector.tensor_tensor(out=xg[:], in0=xn[:],
                                            in1=grep_t[:, lay, :], op=TT.mult)
                    if lay == 0:
                        rps = np_p.tile([128, 128], f32, space="PSUM",
                                        tag="nps", name="rps")
                        nc.tensor.matmul(rps[:], lhsT=xt0_t[:, ts(b, 128)],
                                         rhs=wsk_t[:], start=True, stop=True)
                        xb = qp.tile([128, 128], f32, tag="xb", name="xb")
                        nc.vector.tensor_tensor(out=xb[:], in0=xg[:],
                                                in1=berep_t[:, lay, :],
                                                op=TT.add)
                        xf = qp.tile([128, 128], bf16, tag="xf", name="xf")
                        nc.vector.tensor_tensor(out=xf[:], in0=xb[:],
                                                in1=rps[:], op=TT.add)
                    else:
                        xf = qp.tile([128, 128], bf16, tag="xf", name="xf")
                        nc.vector.tensor_tensor(out=xf[:], in0=xg[:],
                                                in1=berep_t[:, lay, :],
                                                op=TT.add)
                    tps = np_p.tile([128, 128], bf16, space="PSUM",
                                    tag="nps", name="tps")
                    nc.tensor.transpose(tps[:], xf[:], iden_t[:])
                    nc.vector.tensor_copy(xt_next[:, ts(b, 128)], tps[:])

                def emit_chunks(hf, s0, blocks, first, last, oh, rhs_t,
                                sc):
                    for c in range(sc):
                        j = s0 + c
                        b = blocks[j]
                        if first[j]:
                            acc_ps[b] = ap_p.tile(
                                [128, ROWF], f32, space="PSUM", tag="acc",
                                name="acc")
                        nc.tensor.matmul(
                            acc_ps[b][:], lhsT=oh[:, c, :],
                            rhs=rhs_t[:, c, :], start=first[j],
                            stop=last[j], skip_group_check=True)
                        if not last[j]:
                            continue
                        if hf == 0:
                            if "tile" not in acc_a:
                                acc_a["tile"] = asb.tile(
                                    [128, TILES, ROWF], f32, tag="acc_a",
                                    name="acc_a")
                            nc.vector.tensor_copy(
                                acc_a["tile"][:, b, :], acc_ps[b][:])
                            del acc_ps[b]
                            continue
                        tot = qp.tile([128, ROWF], f32, tag="tot", name="tot")
                        nc.vector.tensor_tensor(
                            out=tot[:], in0=acc_a["tile"][:, b, :],
                            in1=acc_ps[b][:], op=TT.add)
                        del acc_ps[b]
                        emit_post(b, tot)

                for hf in range(2):
                    blocks, first, last = _slices_of_half(cbh_list, hf)
                    for s0 in range(0, len(blocks), SLICE_CH):
                        oh, rhs_t, sc = emit_slice(hf, s0, blocks, first,
                                                   last)
                        emit_chunks(hf, s0, blocks, first, last, oh, rhs_t,
                                    sc)
                if lay < 2:
                    xt_cur = xt_next

    # Align gather queue_num with Tile's round-robin DMASW lane assignment
    # (lane i%8 <-> queue i%4 in scheduled Pool order) so each semaphore
    # lane is only ever used by a single SWDGE queue.
    gi = 0
    for bb in nc.main_func.blocks:
        for ins in bb.instructions:
            if isinstance(ins, mybir.InstDMAGatherAnt):
                ins.queue_num = gi % 4
                gi += 1
    nc.compile()
    return nc


def _wrap16(v):
    n = len(v)
    w = v.reshape(n // 16, 16).T  # [16, n/16]
    return np.tile(w, (8, 1)).astype(np.int16)


def _block_diag_att(a):
    # a: [4, C] -> [4*C, 4] block diagonal
    c = a.shape[1]
    out = np.zeros((4 * c, 4), np.float32)
    for h in range(4):
        out[h * c:(h + 1) * c, h] = a[h]
    return out


def prepare_inputs(x, edge_index, W1, as1, ad1, b1, g1, be1, W2, as2, ad2,
                   b2, g2, be2, W3, as3, ad3, b3, Wsk, bsk):
    x = np.asarray(x, np.float32)
    ei = np.asarray(edge_index)
    src = np.concatenate([ei[0], np.arange(N)]).astype(np.int64)
    dst = np.concatenate([ei[1], np.arange(N)]).astype(np.int64)

    score, sloc = src // NLOC, src % NLOC
    half = (sloc >= A_LOC).astype(np.int64)
    gidx = np.where(half == 0, A_LOC * score + sloc,
                    B_LOC * score + (sloc - A_LOC))
    dcore, dloc = dst // NLOC, dst % NLOC
    blk = dloc >> 7
    doff = dloc & 127

    # per-(block, half) chunk counts: max over cores (SPMD-identical program)
    gid = (dcore * TILES + blk) * 2 + half
    counts = np.bincount(gid, minlength=NCORES * TILES * 2).reshape(
        NCORES, TILES, 2)
    cbh = np.maximum(1, -(-counts.max(axis=0) // 128))  # [TILES, 2]
    cbh_list = tuple(tuple(int(v) for v in row) for row in cbh)

    # chunk offset of each (block, half) within its half's chunk sequence
    off = np.zeros((TILES, 2), np.int64)
    for h in range(2):
        off[:, h] = np.concatenate([[0], np.cumsum(cbh[:, h])[:-1]])
    ch_half = [int(cbh[:, h].sum()) for h in range(2)]

    in_maps = []
    wext = np.stack([
        np.concatenate([
            np.asarray(W, np.float32),
            np.asarray(W, np.float32) @ _block_diag_att(
                np.asarray(a_s, np.float32)),
            np.asarray(W, np.float32) @ _block_diag_att(
                np.asarray(a_d, np.float32))], axis=1)
        for (W, a_s, a_d) in [(W1, as1, ad1), (W2, as2, ad2), (W3, as3, ad3)]
    ]).astype(ml_dtypes.bfloat16)
    brep = np.stack([np.broadcast_to(np.asarray(b1, np.float32), (128, 128)),
                     np.broadcast_to(np.asarray(b2, np.float32), (128, 128))])
    grep = np.stack([np.broadcast_to(np.asarray(g1, np.float32), (128, 128)),
                     np.broadcast_to(np.asarray(g2, np.float32), (128, 128))])
    berep = np.stack([
        np.broadcast_to(np.asarray(be1, np.float32)
                        + np.asarray(bsk, np.float32), (128, 128)),
        np.broadcast_to(np.asarray(be2, np.float32), (128, 128))])
    b3rep = np.ascontiguousarray(
        np.broadcast_to(np.asarray(b3, np.float32), (128, 32)))
    iota = np.tile(np.arange(128, dtype=np.float32),
                   (128, 1)).astype(ml_dtypes.bfloat16)
    iden = np.eye(128, dtype=np.float32).astype(ml_dtypes.bfloat16)
    wsk = np.asarray(Wsk, np.float32).astype(ml_dtypes.bfloat16)

    for c in range(NCORES):
        m = dcore == c
        e_blk, e_half, e_gidx, e_dloc, e_doff = (
            blk[m], half[m], gidx[m], dloc[m], doff[m])
        core_map = {}
        for hf in range(2):
            mh = e_half == hf
            b_, g_, l_, d_ = (e_blk[mh], e_gidx[mh], e_dloc[mh], e_doff[mh])
            order = np.lexsort((g_, b_))
            b_, g_, l_, d_ = b_[order], g_[order], l_[order], d_[order]
            starts = np.searchsorted(b_, np.arange(TILES))
            rank = np.arange(len(b_)) - starts[b_]
            pos = off[b_, hf] * 128 + rank
            ne = ch_half[hf] * 128
            gi = np.zeros(ne, np.int64)
            gl = np.zeros(ne, np.int64)
            gd = np.full(ne, -1.0, np.float64)
            gi[pos] = g_
            gl[pos] = l_
            gd[pos] = d_
            idx2 = np.stack([_wrap16(gi), _wrap16(gl)], axis=1)
            core_map[f"IDX2{hf}"] = idx2
            core_map[f"DOFF{hf}"] = np.ascontiguousarray(
                gd.reshape(-1, 128).T).astype(ml_dtypes.bfloat16)
        xt = np.zeros((128, NSH), np.float32)
        xt[:, :NLOC] = x[c * NLOC:(c + 1) * NLOC].T
        core_map.update({
            "XT0": xt.astype(ml_dtypes.bfloat16), "WEXT": wext, "WSK": wsk,
            "BREP": brep, "GREP": grep, "BEREP": berep, "B3REP": b3rep,
            "IOTA": iota, "IDEN": iden,
        })
        in_maps.append(core_map)

    return in_maps, cbh_list


def kernel(**inputs):
    global LAST_EXEC_NS, LAST_RESULTS
    in_maps, cbh_list = prepare_inputs(**inputs)
    if cbh_list not in _PROGRAM_CACHE:
        _PROGRAM_CACHE[cbh_list] = _build_program(cbh_list)
    nc = _PROGRAM_CACHE[cbh_list]

    if TRACE:
        _install_ntff_hook()
    res = run_bass_kernel_spmd(nc, in_maps, list(range(NCORES)),
                               trace=TRACE)
    LAST_EXEC_NS = res.exec_time_ns
    LAST_RESULTS = res
    out = np.concatenate(
        [res.results[c]["OUT"][:NLOC] for c in range(NCORES)], axis=0)
    return out.astype(np.float32)
